# revision 2
# baseline (speedup 1.0000x reference)
"""Causal multi-head attention on 8 Trainium2 NeuronCores.

Problem: x[2,2048,1024] @ W_Q/K/V[1024,1024] -> 16-head causal attention
(d_head=64) -> @ W_O[1024,1024].

Sharding: tensor-parallel over heads. Core i owns heads 2i, 2i+1 — i.e.
columns [128i:128i+128) of W_Q/W_K/W_V and rows [128i:128i+128) of W_O.
Each core computes its partial output [1024, 4096] (transposed layout);
the host sums the 8 partials and un-transposes (the "all-reduce").

Device kernel (per core, all matmuls in float32r = full-rate fp32):
  1. Projections from xT [1024, 4096] (host pre-transposes x):
     QT/KT [128, 4096] = W.T @ xT; V in natural [token, dim] layout via
     PE transpose, with a ones-column appended per head (65-wide blocks)
     so the PV matmul also produces the softmax denominator for free.
  2. Flash-style causal attention with scores in [k, q] orientation:
     scoresT = KT.T-slice @ QT-slice, exp on ScalarE (no max-subtraction:
     scores ~ N(0,1), exp is safe in fp32), causal mask applied
     multiplicatively on the 4 diagonal chunk variants only, PV matmul
     accumulates [65, 512] (64 dims + denominator row) in PSUM.
  3. Normalize by the denominator row (reciprocal + partition broadcast),
     then outT_partial = W_O-slice.T @ attnT.
"""

import numpy as np

import concourse.bass as bass
import concourse.tile as tile
from concourse import bacc, mybir
from concourse.bass_utils import run_bass_kernel_spmd
from concourse.masks import make_identity

F32 = mybir.dt.float32
F32R = mybir.dt.float32r

N_CORES = 8
P = 128
D = 1024          # d_model
B = 2             # batch
S = 2048          # seq len
T = B * S         # total tokens = 4096
TT = 512          # token tile (free dim of matmuls)
NT = T // TT      # 8 token tiles
KD = D // P       # 8 contraction chunks for projections
JB = S // TT      # 4 q-tiles per batch
CB = S // P       # 16 k-chunks per batch
NCH = T // P      # 32 k-chunks total
H_LOC = 2         # heads per core
DH = 64           # head dim


def _body(tc):
    nc = tc.nc
    xT = nc.dram_tensor("xT", [D, T], F32R, kind="ExternalInput").ap()
    wq = nc.dram_tensor("wq", [D, P], F32R, kind="ExternalInput").ap()
    wk = nc.dram_tensor("wk", [D, P], F32R, kind="ExternalInput").ap()
    wv = nc.dram_tensor("wv", [D, P], F32R, kind="ExternalInput").ap()
    wo = nc.dram_tensor("wo", [P, D], F32R, kind="ExternalInput").ap()
    outT = nc.dram_tensor("outT", [D, T], F32, kind="ExternalOutput").ap()

    import contextlib
    with contextlib.ExitStack() as ctx:
        const = ctx.enter_context(tc.tile_pool(name="const", bufs=1))
        wpool = ctx.enter_context(tc.tile_pool(name="wpool", bufs=1))
        xpool = ctx.enter_context(tc.tile_pool(name="xpool", bufs=2))
        persist = ctx.enter_context(tc.tile_pool(name="persist", bufs=1))
        probs_p = ctx.enter_context(tc.tile_pool(name="probs", bufs=10))
        stage = ctx.enter_context(tc.tile_pool(name="stage", bufs=3))
        psum = ctx.enter_context(tc.tile_pool(name="psum", bufs=4, space="PSUM"))

        # --- constants -----------------------------------------------------
        identity = const.tile([P, P], F32)
        make_identity(nc, identity)

        # mask_band[k, q] = 1.0 if q >= k else 0.0 (lower-left triangular 0s)
        mask_band = const.tile([P, P], F32)
        nc.any.memset(mask_band[:], 1.0)
        nc.gpsimd.affine_select(
            out=mask_band[:],
            in_=mask_band[:],
            compare_op=mybir.AluOpType.is_ge,
            fill=0.0,
            base=0,
            pattern=[[1, P]],
            channel_multiplier=-1,
        )

        # --- weights -------------------------------------------------------
        wq_sb = wpool.tile([P, KD, P], F32R)
        nc.sync.dma_start(wq_sb[:], wq.rearrange("(o p) m -> p o m", p=P))
        wk_sb = wpool.tile([P, KD, P], F32R)
        nc.sync.dma_start(wk_sb[:], wk.rearrange("(o p) m -> p o m", p=P))
        wv_sb = wpool.tile([P, KD, P], F32R)
        nc.sync.dma_start(wv_sb[:], wv.rearrange("(o p) m -> p o m", p=P))
        wo_sb = wpool.tile([P, D], F32R)
        nc.sync.dma_start(wo_sb[:], wo)

        # --- persistent activations ---------------------------------------
        qT = persist.tile([P, T], F32R)       # [2h x 64d, tokens]
        kT = persist.tile([P, T], F32R)
        vn = persist.tile([P, NCH, 130], F32R)  # [token, chunk, d0|1|d1|1]
        attnT = persist.tile([P, T], F32R)
        for col in (DH, 2 * DH + 1):
            nc.scalar.activation(vn[:, :, col], vn[:, :, col],
                                 mybir.ActivationFunctionType.Identity,
                                 bias=1.0, scale=0.0)

        xT_r = xT.rearrange("(o p) n -> p o n", p=P)
        outT_r = outT.rearrange("(o p) n -> p o n", p=P)

        # --- phase 1: projections -----------------------------------------
        for t in range(NT):
            xt = xpool.tile([P, KD, TT], F32R)
            for c in range(KD):
                nc.sync.dma_start(xt[:, c, :], xT_r[:, c, bass.ts(t, TT)])
            for wsb, dstT in ((wq_sb, qT), (wk_sb, kT)):
                ps = psum.tile([P, TT], F32, tag="a")
                for c in range(KD):
                    nc.tensor.matmul(ps[:], wsb[:, c, :], xt[:, c, :],
                                     start=(c == 0), stop=(c == KD - 1))
                nc.vector.tensor_copy(dstT[:, bass.ts(t, TT)], ps[:])
            # V: project, then PE-transpose into natural [token, dim] layout
            ps = psum.tile([P, TT], F32, tag="a")
            for c in range(KD):
                nc.tensor.matmul(ps[:], wv_sb[:, c, :], xt[:, c, :],
                                 start=(c == 0), stop=(c == KD - 1))
            vt = stage.tile([P, TT], F32, tag="vt")
            nc.vector.tensor_copy(vt[:], ps[:])
            for s_ in range(4):
                pt = psum.tile([P, P], F32, tag="b")
                nc.tensor.transpose(pt[:], vt[:, bass.ts(s_, P)], identity)
                ch = t * 4 + s_
                nc.vector.tensor_copy(vn[:, ch, 0:DH], pt[:, 0:DH])
                nc.vector.tensor_copy(vn[:, ch, DH + 1:2 * DH + 1],
                                      pt[:, DH:2 * DH])

        # --- phase 2: causal attention ------------------------------------
        # Dual-j: the same-index q-tiles of batch 0 and batch 1 are
        # processed together (same causal shape), doubling the independent
        # matmul streams in flight. Lag-1 software pipeline: the PV matmul
        # for chunk cb-1 is emitted after the scores matmul for chunk cb so
        # the PE never waits on ScalarE's exp. Diagonal chunk r: exp/PV only
        # the live columns [128r:], triangular mask on the 128-wide band.
        for jj in range(JB):
            ncb = 4 * (jj + 1)
            js = (jj, jj + JB)
            pvs = {}
            for jx in js:
                for h in range(H_LOC):
                    pvs[(jx, h)] = psum.tile([DH + 1, TT], F32, tag="a",
                                             name=f"pv_{jx}_{h}")

            def pv_step(jx, cb, prs, jj=jj, ncb=ncb, pvs=pvs):
                b = jx // JB
                c = CB * b + cb
                r = cb - 4 * jj
                lo = P * r if r > 0 else 0
                for h in range(H_LOC):
                    nc.tensor.matmul(pvs[(jx, h)][:, lo:],
                                     vn[:, c, bass.ds((DH + 1) * h, DH + 1)],
                                     prs[h][:, lo:],
                                     start=(cb == 0), stop=(cb == ncb - 1))

            pending = {}
            for cb in range(ncb):
                r = cb - 4 * jj
                lo = P * r if r > 0 else 0
                for jx in js:
                    b = jx // JB
                    c = CB * b + cb
                    csl = bass.ts(c, P)
                    jsl = bass.ts(jx, TT)
                    prs = []
                    for h in range(H_LOC):
                        hp = slice(DH * h, DH * h + DH)
                        sps = psum.tile([P, TT], F32, tag="b",
                                        name=f"sps_{jx}_{cb}_{h}")
                        nc.tensor.matmul(sps[:, lo:], kT[hp, csl],
                                         qT[hp, jsl][:, lo:],
                                         start=True, stop=True)
                        pr = probs_p.tile([P, TT], F32R, tag="pr",
                                          name=f"pr_{jx}_{cb}_{h}")
                        nc.scalar.activation(pr[:, lo:], sps[:, lo:],
                                             mybir.ActivationFunctionType.Exp,
                                             scale=0.125)
                        if r >= 0:
                            nc.vector.tensor_mul(pr[:, bass.ts(r, P)],
                                                 pr[:, bass.ts(r, P)],
                                                 mask_band[:])
                        prs.append(pr)
                    if jx in pending:
                        pv_step(jx, cb - 1, pending[jx])
                    pending[jx] = prs
            for jx in js:
                pv_step(jx, ncb - 1, pending[jx])

            for jx in js:
                jsl = bass.ts(jx, TT)
                for h in range(H_LOC):
                    hp = slice(DH * h, DH * h + DH)
                    rc = stage.tile([1, TT], F32, tag="rc",
                                    name=f"rc_{jx}_{h}")
                    nc.vector.reciprocal(rc[:], pvs[(jx, h)][DH:DH + 1, :])
                    rb = stage.tile([DH, TT], F32, tag="rb",
                                    name=f"rb_{jx}_{h}")
                    nc.gpsimd.partition_broadcast(rb[:], rc[:])
                    nc.vector.tensor_mul(attnT[hp, jsl],
                                         pvs[(jx, h)][0:DH, :], rb[:])

        # --- phase 3: output projection (partial) -------------------------
        for j in range(NT):
            for f in range(KD):
                wps = psum.tile([P, TT], F32, tag="b", name=f"wps_{j}_{f}")
                nc.tensor.matmul(wps[:], wo_sb[:, bass.ts(f, P)],
                                 attnT[:, bass.ts(j, TT)],
                                 start=True, stop=True)
                ob = stage.tile([P, TT], F32, tag="ob", name=f"ob_{j}_{f}")
                nc.vector.tensor_copy(ob[:], wps[:])
                nc.sync.dma_start(outT_r[:, f, bass.ts(j, TT)], ob[:])


_NC_CACHE = None


def _get_nc():
    global _NC_CACHE
    if _NC_CACHE is None:
        nc = bacc.Bacc("TRN2", target_bir_lowering=False, debug=False,
                       num_devices=N_CORES)
        with tile.TileContext(nc) as tc:
            _body(tc)
        nc.compile()
        _NC_CACHE = nc
    return _NC_CACHE


def _in_maps(x, W_Q, W_K, W_V, W_O):
    xT = np.ascontiguousarray(
        np.asarray(x, dtype=np.float32).reshape(T, D).T)
    W_Q = np.asarray(W_Q, dtype=np.float32)
    W_K = np.asarray(W_K, dtype=np.float32)
    W_V = np.asarray(W_V, dtype=np.float32)
    W_O = np.asarray(W_O, dtype=np.float32)
    maps = []
    for i in range(N_CORES):
        sl = slice(P * i, P * i + P)
        maps.append({
            "xT": xT,
            "wq": np.ascontiguousarray(W_Q[:, sl]),
            "wk": np.ascontiguousarray(W_K[:, sl]),
            "wv": np.ascontiguousarray(W_V[:, sl]),
            "wo": np.ascontiguousarray(W_O[sl, :]),
        })
    return maps


def _gather(results):
    acc = np.zeros([D, T], np.float64)
    for r in results:
        acc += r["outT"]
    return np.ascontiguousarray(
        acc.T.astype(np.float32)).reshape(B, S, D)


def kernel(x, W_Q, W_K, W_V, W_O):
    nc = _get_nc()
    res = run_bass_kernel_spmd(nc, _in_maps(x, W_Q, W_K, W_V, W_O),
                               core_ids=list(range(N_CORES)))
    return _gather(res.results)


def kernel_profiled(x, W_Q, W_K, W_V, W_O):
    """Like kernel() but with NTFF tracing.

    Returns (output, exec_time_ns, insts) — insts is the annotated
    gauge instruction list for the traced core (or None).
    """
    nc = _get_nc()
    res = run_bass_kernel_spmd(nc, _in_maps(x, W_Q, W_K, W_V, W_O),
                               core_ids=list(range(N_CORES)), trace=True)
    insts = None
    if res.instructions_and_trace is not None:
        insts = res.instructions_and_trace[0]
    return _gather(res.results), res.exec_time_ns, insts



# revision 9
# speedup vs baseline: 1.7945x; 1.7945x over previous
"""Causal multi-head attention on 8 Trainium2 NeuronCores.

Problem: x[2,2048,1024] @ W_Q/K/V[1024,1024] -> 16-head causal attention
(d_head=64) -> @ W_O[1024,1024].

Sharding: tensor-parallel over heads. Core i owns heads 2i, 2i+1 — i.e.
columns [128i:128i+128) of W_Q/W_K/W_V and rows [128i:128i+128) of W_O.
Each core computes its partial output [1024, 4096] (transposed layout,
bf16); the host sums the 8 partials in f32 and un-transposes (the
"all-reduce").

v2 (this file): all-bf16 dataflow tuned for PE occupancy.
  - All matmul operands bf16 (1 cyc/row incl. narrow tiles; fast
    LDWEIGHTS so weight loads hide under matmuls), PSUM accumulates f32.
  - Scores for both heads of a (q-tile, k-chunk) land in one 2-bank
    PSUM unit -> ONE ScalarE exp instruction for both heads (halves
    Activation instruction overhead; ScalarE is the phase-2 co-wall).
  - Softmax denominator via a ones-column in the V tile (PV matmul row
    64), normalized with reciprocal_approx_fast + stride-0 partition
    broadcast (the old [1,512] nc.vector.reciprocal was 3.3us each).
  - W_O matmuls + output DMA are spread through the NEXT q-tile's
    attention loop so the PE never idles at tile boundaries and the
    16.8MB->8.4MB output writeback overlaps compute.
  - Input x, all weights, output: bf16 on the wire (halves HBM traffic;
    rel-err gate is 2e-2, measured ~1e-3).
"""

import contextlib

import ml_dtypes
import numpy as np

import concourse.bass as bass
import concourse.tile as tile
from concourse import bacc, mybir
from concourse.bass_utils import run_bass_kernel_spmd
from concourse.masks import make_identity

F32 = mybir.dt.float32
BF16 = mybir.dt.bfloat16
EXP = mybir.ActivationFunctionType.Exp

N_CORES = 8
P = 128
D = 1024          # d_model
B = 2             # batch
S = 2048          # seq len
T = B * S         # total tokens = 4096
TT = 512          # token tile (free dim of matmuls)
NT = T // TT      # 8 token tiles
KD = D // P       # 8 contraction chunks for projections
JB = S // TT      # 4 q-tiles per batch
CB = S // P       # 16 k-chunks per batch
NCH = T // P      # 32 k-chunks total
H_LOC = 2         # heads per core
DH = 64           # head dim


DEBUG_DUMP = False


def _body(tc):
    nc = tc.nc
    xT = nc.dram_tensor("xT", [D, T], BF16, kind="ExternalInput").ap()
    wq = nc.dram_tensor("wq", [D, P], BF16, kind="ExternalInput").ap()
    wk = nc.dram_tensor("wk", [D, P], BF16, kind="ExternalInput").ap()
    wv = nc.dram_tensor("wv", [D, P], BF16, kind="ExternalInput").ap()
    wo = nc.dram_tensor("wo", [P, D], BF16, kind="ExternalInput").ap()
    outT = nc.dram_tensor("outT", [D, T], BF16, kind="ExternalOutput").ap()

    xT_r = xT.rearrange("(o p) n -> p o n", p=P)
    outT_r = outT.rearrange("(o p) n -> p o n", p=P)

    with contextlib.ExitStack() as ctx:
        const = ctx.enter_context(tc.tile_pool(name="const", bufs=1))
        wpool = ctx.enter_context(tc.tile_pool(name="wpool", bufs=1))
        xpool = ctx.enter_context(tc.tile_pool(name="xpool", bufs=2))
        persist = ctx.enter_context(tc.tile_pool(name="persist", bufs=1))
        prp = ctx.enter_context(tc.tile_pool(name="probs", bufs=6))
        stage = ctx.enter_context(tc.tile_pool(name="stage", bufs=2))
        obp = ctx.enter_context(tc.tile_pool(name="obp", bufs=3))
        psum = ctx.enter_context(tc.tile_pool(name="psum", bufs=2, space="PSUM"))

        # --- constants -----------------------------------------------------
        identity = const.tile([P, P], BF16)
        make_identity(nc, identity)

        # mask_band[k, q] = 1.0 if q >= k else 0.0
        mask_band = const.tile([P, P], BF16)
        nc.gpsimd.memset(mask_band[:], 1.0)
        nc.gpsimd.affine_select(
            out=mask_band[:],
            in_=mask_band[:],
            compare_op=mybir.AluOpType.is_ge,
            fill=0.0,
            base=0,
            pattern=[[1, P]],
            channel_multiplier=-1,
        )

        # --- weights (scalar-engine DMA queue; x tiles own the sync queue) -
        wq_sb = wpool.tile([P, KD, P], BF16)
        nc.scalar.dma_start(wq_sb[:], wq.rearrange("(o p) m -> p o m", p=P))
        wk_sb = wpool.tile([P, KD, P], BF16)
        nc.scalar.dma_start(wk_sb[:], wk.rearrange("(o p) m -> p o m", p=P))
        wv_sb = wpool.tile([P, KD, P], BF16)
        nc.scalar.dma_start(wv_sb[:], wv.rearrange("(o p) m -> p o m", p=P))
        wo_sb = wpool.tile([P, D], BF16)
        nc.scalar.dma_start(wo_sb[:], wo)

        # --- persistent activations ---------------------------------------
        qT = persist.tile([P, T], BF16)       # [2h x 64d, tokens]
        kT = persist.tile([P, T], BF16)
        vn = persist.tile([P, NCH, 2 * DH + 2], BF16)  # [tok, chunk, d0|1|d1|1]
        attnT = persist.tile([P, T], BF16)
        nc.gpsimd.memset(vn[:, :, DH], 1.0)
        nc.gpsimd.memset(vn[:, :, 2 * DH + 1], 1.0)

        # --- phase 1: projections -----------------------------------------
        for t in range(NT):
            xt = xpool.tile([P, KD, TT], BF16, name=f"xt_{t}")
            nc.sync.dma_start(xt[:], xT_r[:, :, bass.ts(t, TT)])
            uqk = psum.tile([P, 2, TT], F32, tag="sc", name=f"uqk_{t}")
            for k, wsb in ((0, wq_sb), (1, wk_sb)):
                for c in range(KD):
                    nc.tensor.matmul(uqk[:, k, :], wsb[:, c, :], xt[:, c, :],
                                     start=(c == 0), stop=(c == KD - 1))
            nc.vector.tensor_copy(qT[:, bass.ts(t, TT)], uqk[:, 0, :])
            nc.vector.tensor_copy(kT[:, bass.ts(t, TT)], uqk[:, 1, :])
            # V: project, then PE-transpose into natural [token, dim] layout
            uv = psum.tile([P, 2, TT], F32, tag="sc", name=f"uv_{t}")
            for c in range(KD):
                nc.tensor.matmul(uv[:, 0, :], wv_sb[:, c, :], xt[:, c, :],
                                 start=(c == 0), stop=(c == KD - 1))
            vt = stage.tile([P, TT], BF16, tag="vt", name=f"vt_{t}")
            nc.vector.tensor_copy(vt[:], uv[:, 0, :])
            pt = psum.tile([P, 4, P], BF16, tag="sc", name=f"pt_{t}")
            for s_ in range(4):
                nc.tensor.transpose(pt[:, s_, :], vt[:, bass.ts(s_, P)],
                                    identity)
            # one strided copy drops both heads' dims around the ones cols
            dst = vn[:, bass.ts(t, 4), :].rearrange("p c (h k) -> p c h k",
                                                    h=2)[:, :, :, 0:DH]
            src = pt[:].rearrange("p c (h k) -> p c h k", h=2)
            nc.vector.tensor_copy(dst, src)

        # --- phase 2: causal attention + interleaved output projection ----
        def pv_step(pvs, jj, ncb, jx, cb, pr):
            b = jx // JB
            c = CB * b + cb
            r = cb - 4 * jj
            lo = P * r if r > 0 else 0
            for h in range(H_LOC):
                nc.tensor.matmul(pvs[(jx, h)][:, lo:],
                                 vn[:, c, bass.ds((DH + 1) * h, DH + 1)],
                                 pr[:, h, lo:],
                                 start=(cb == 0), stop=(cb == ncb - 1))

        def emit_normalize(js, pvs):
            for jx in js:
                jsl = bass.ts(jx, TT)
                for h in range(H_LOC):
                    # custom-DVE recip reads garbage from PSUM; stage to SBUF
                    dn = stage.tile([1, TT], F32, tag="dn",
                                    name=f"dn_{jx}_{h}")
                    nc.vector.tensor_copy(dn[:], pvs[(jx, h)][DH:DH + 1, :])
                    rc = stage.tile([1, TT], F32, tag="rc",
                                    name=f"rc_{jx}_{h}")
                    nc.vector.reciprocal_approx_fast(rc[:], dn[:])
                    rb = stage.tile([DH, TT], F32, tag="rb",
                                    name=f"rb_{jx}_{h}")
                    nc.gpsimd.partition_broadcast(rb[:], rc[:])
                    nc.vector.tensor_mul(
                        attnT[bass.ds(DH * h, DH), jsl],
                        pvs[(jx, h)][0:DH, :], rb[:])

        def emit_wo_unit(jx, fi):
            jsl = bass.ts(jx, TT)
            u = psum.tile([P, 2, TT], F32, tag="sc", name=f"wo_{jx}_{fi}")
            for k in range(2):
                nc.tensor.matmul(u[:, k, :], wo_sb[:, bass.ts(2 * fi + k, P)],
                                 attnT[:, jsl], start=True, stop=True)
            ob = obp.tile([P, 2, TT], BF16, tag="ob", name=f"ob_{jx}_{fi}")
            nc.vector.tensor_copy(ob[:], u[:])
            nc.sync.dma_start(outT_r[:, 2 * fi:2 * fi + 2, jsl], ob[:])

        prev = None  # (js, pvs) of the previous q-tile pair
        for jj in range(JB):
            ncb = 4 * (jj + 1)
            js = (jj, jj + JB)
            wo_work = []
            if prev is not None:
                emit_normalize(*prev)
                wo_work = [(jx, fi) for jx in prev[0] for fi in range(4)]
            pvs = {}
            for jx in js:
                for h in range(H_LOC):
                    pvs[(jx, h)] = psum.tile([DH + 1, TT], F32, tag="a",
                                             bufs=4, name=f"pv_{jx}_{h}")

            pend = {}
            for cb in range(ncb):
                r = cb - 4 * jj
                lo = P * r if r > 0 else 0
                for jx in js:
                    b = jx // JB
                    c = CB * b + cb
                    u = psum.tile([P, 2, TT], F32, tag="sc",
                                  name=f"sc_{jx}_{cb}")
                    for h in range(H_LOC):
                        hp = slice(DH * h, DH * h + DH)
                        nc.tensor.matmul(u[:, h, lo:], kT[hp, bass.ts(c, P)],
                                         qT[hp, bass.ts(jx, TT)][:, lo:],
                                         start=True, stop=True)
                    pr = prp.tile([P, 2, TT], BF16, tag="pr",
                                  name=f"pr_{jx}_{cb}")
                    nc.scalar.activation(pr[:, :, lo:], u[:, :, lo:], EXP,
                                         scale=0.125)
                    if r >= 0:
                        for h in range(H_LOC):
                            nc.vector.tensor_mul(pr[:, h, bass.ts(r, P)],
                                                 pr[:, h, bass.ts(r, P)],
                                                 mask_band[:])
                    if jx in pend:
                        pv_step(pvs, jj, ncb, jx, *pend[jx])
                    pend[jx] = (cb, pr)
                if cb >= 1:
                    for _ in range(2):
                        if wo_work:
                            emit_wo_unit(*wo_work.pop(0))
            for jx in js:
                pv_step(pvs, jj, ncb, jx, *pend.pop(jx))
            while wo_work:
                emit_wo_unit(*wo_work.pop(0))
            prev = (js, pvs)

        # --- tail: last q-tile pair's normalize + output projection -------
        emit_normalize(*prev)
        for jx in prev[0]:
            for fi in range(4):
                emit_wo_unit(jx, fi)

        if DEBUG_DUMP:
            dq = nc.dram_tensor("dbg_qT", [P, T], BF16,
                                kind="ExternalOutput").ap()
            dk = nc.dram_tensor("dbg_kT", [P, T], BF16,
                                kind="ExternalOutput").ap()
            dv = nc.dram_tensor("dbg_vn", [P, NCH, 2 * DH + 2], BF16,
                                kind="ExternalOutput").ap()
            da = nc.dram_tensor("dbg_attnT", [P, T], BF16,
                                kind="ExternalOutput").ap()
            nc.sync.dma_start(dq, qT[:])
            nc.sync.dma_start(dk, kT[:])
            nc.sync.dma_start(dv, vn[:])
            nc.sync.dma_start(da, attnT[:])


_NC_CACHE = None


def _get_nc():
    global _NC_CACHE
    if _NC_CACHE is None:
        nc = bacc.Bacc("TRN2", target_bir_lowering=False, debug=False,
                       num_devices=N_CORES)
        with tile.TileContext(nc) as tc:
            _body(tc)
        nc.compile()
        _NC_CACHE = nc
    return _NC_CACHE


_BF = ml_dtypes.bfloat16


def _in_maps(x, W_Q, W_K, W_V, W_O):
    xT = np.ascontiguousarray(
        np.asarray(x, dtype=np.float32).reshape(T, D).T).astype(_BF)
    W_Q = np.asarray(W_Q, dtype=np.float32)
    W_K = np.asarray(W_K, dtype=np.float32)
    W_V = np.asarray(W_V, dtype=np.float32)
    W_O = np.asarray(W_O, dtype=np.float32)
    maps = []
    for i in range(N_CORES):
        sl = slice(P * i, P * i + P)
        maps.append({
            "xT": xT,
            "wq": np.ascontiguousarray(W_Q[:, sl]).astype(_BF),
            "wk": np.ascontiguousarray(W_K[:, sl]).astype(_BF),
            "wv": np.ascontiguousarray(W_V[:, sl]).astype(_BF),
            "wo": np.ascontiguousarray(W_O[sl, :]).astype(_BF),
        })
    return maps


def _gather(results):
    acc = np.zeros([D, T], np.float32)
    for r in results:
        acc += r["outT"].astype(np.float32)
    return np.ascontiguousarray(acc.T).reshape(B, S, D)


def kernel(x, W_Q, W_K, W_V, W_O):
    nc = _get_nc()
    res = run_bass_kernel_spmd(nc, _in_maps(x, W_Q, W_K, W_V, W_O),
                               core_ids=list(range(N_CORES)))
    return _gather(res.results)


def kernel_profiled(x, W_Q, W_K, W_V, W_O):
    """Like kernel() but with NTFF tracing.

    Returns (output, exec_time_ns, insts) — insts is the annotated
    gauge instruction list for the traced core (or None).
    """
    nc = _get_nc()
    res = run_bass_kernel_spmd(nc, _in_maps(x, W_Q, W_K, W_V, W_O),
                               core_ids=list(range(N_CORES)), trace=True)
    insts = None
    if res.instructions_and_trace is not None:
        insts = res.instructions_and_trace[0]
    return _gather(res.results), res.exec_time_ns, insts


# revision 13
# speedup vs baseline: 1.8375x; 1.0240x over previous
"""Causal multi-head attention on 8 Trainium2 NeuronCores.

Problem: x[2,2048,1024] @ W_Q/K/V[1024,1024] -> 16-head causal attention
(d_head=64) -> @ W_O[1024,1024].

Sharding: tensor-parallel over heads. Core i owns heads 2i, 2i+1 — i.e.
columns [128i:128i+128) of W_Q/W_K/W_V and rows [128i:128i+128) of W_O.
Each core computes its partial output [1024, 4096] (transposed layout,
bf16); the host sums the 8 partials in f32 and un-transposes (the
"all-reduce").

v2 (this file): all-bf16 dataflow tuned for PE occupancy.
  - All matmul operands bf16 (1 cyc/row incl. narrow tiles; fast
    LDWEIGHTS so weight loads hide under matmuls), PSUM accumulates f32.
  - Scores for both heads of a (q-tile, k-chunk) land in one 2-bank
    PSUM unit -> ONE ScalarE exp instruction for both heads (halves
    Activation instruction overhead; ScalarE is the phase-2 co-wall).
  - Softmax denominator via a ones-column in the V tile (PV matmul row
    64), normalized with reciprocal_approx_fast + stride-0 partition
    broadcast (the old [1,512] nc.vector.reciprocal was 3.3us each).
  - W_O matmuls + output DMA are spread through the NEXT q-tile's
    attention loop so the PE never idles at tile boundaries and the
    16.8MB->8.4MB output writeback overlaps compute.
  - Input x, all weights, output: bf16 on the wire (halves HBM traffic;
    rel-err gate is 2e-2, measured ~1e-3).
"""

import contextlib

import ml_dtypes
import numpy as np

import concourse.bass as bass
import concourse.tile as tile
from concourse import bacc, mybir
from concourse.bass_utils import run_bass_kernel_spmd
from concourse.masks import make_identity

F32 = mybir.dt.float32
BF16 = mybir.dt.bfloat16
EXP = mybir.ActivationFunctionType.Exp

N_CORES = 8
P = 128
D = 1024          # d_model
B = 2             # batch
S = 2048          # seq len
T = B * S         # total tokens = 4096
TT = 512          # token tile (free dim of matmuls)
NT = T // TT      # 8 token tiles
KD = D // P       # 8 contraction chunks for projections
JB = S // TT      # 4 q-tiles per batch
CB = S // P       # 16 k-chunks per batch
NCH = T // P      # 32 k-chunks total
H_LOC = 2         # heads per core
DH = 64           # head dim


DEBUG_DUMP = False


def _body(tc):
    nc = tc.nc
    xT = nc.dram_tensor("xT", [D, T], BF16, kind="ExternalInput").ap()
    wq = nc.dram_tensor("wq", [D, P], BF16, kind="ExternalInput").ap()
    wk = nc.dram_tensor("wk", [D, P], BF16, kind="ExternalInput").ap()
    wv = nc.dram_tensor("wv", [D, P], BF16, kind="ExternalInput").ap()
    wo = nc.dram_tensor("wo", [P, D], BF16, kind="ExternalInput").ap()
    outT = nc.dram_tensor("outT", [D, T], BF16, kind="ExternalOutput").ap()

    xT_r = xT.rearrange("(o p) n -> p o n", p=P)
    outT_r = outT.rearrange("(o p) n -> p o n", p=P)

    with contextlib.ExitStack() as ctx:
        const = ctx.enter_context(tc.tile_pool(name="const", bufs=1))
        wpool = ctx.enter_context(tc.tile_pool(name="wpool", bufs=1))
        xpool = ctx.enter_context(tc.tile_pool(name="xpool", bufs=2))
        persist = ctx.enter_context(tc.tile_pool(name="persist", bufs=1))
        prp = ctx.enter_context(tc.tile_pool(name="probs", bufs=6))
        stage = ctx.enter_context(tc.tile_pool(name="stage", bufs=2))
        obp = ctx.enter_context(tc.tile_pool(name="obp", bufs=4))
        psum = ctx.enter_context(tc.tile_pool(name="psum", bufs=2, space="PSUM"))

        # --- constants -----------------------------------------------------
        identity = const.tile([P, P], BF16)
        make_identity(nc, identity)

        # mask_band[k, q] = 1.0 if q >= k else 0.0
        mask_band = const.tile([P, P], BF16)
        nc.gpsimd.memset(mask_band[:], 1.0)
        nc.gpsimd.affine_select(
            out=mask_band[:],
            in_=mask_band[:],
            compare_op=mybir.AluOpType.is_ge,
            fill=0.0,
            base=0,
            pattern=[[1, P]],
            channel_multiplier=-1,
        )

        # --- weights (scalar-engine DMA queue; x tiles own the sync queue) -
        wq_sb = wpool.tile([P, KD, P], BF16)
        nc.scalar.dma_start(wq_sb[:], wq.rearrange("(o p) m -> p o m", p=P))
        wk_sb = wpool.tile([P, KD, P], BF16)
        nc.scalar.dma_start(wk_sb[:], wk.rearrange("(o p) m -> p o m", p=P))
        wv_sb = wpool.tile([P, KD, P], BF16)
        nc.scalar.dma_start(wv_sb[:], wv.rearrange("(o p) m -> p o m", p=P))
        wo_sb = wpool.tile([P, D], BF16)
        nc.scalar.dma_start(wo_sb[:], wo)

        # --- persistent activations ---------------------------------------
        qT = persist.tile([P, T], BF16)       # [2h x 64d, tokens]
        kT = persist.tile([P, T], BF16)
        vn = persist.tile([P, NCH, 2 * DH + 2], BF16)  # [tok, chunk, d0|1|d1|1]
        attnT = persist.tile([P, T], BF16)
        nc.gpsimd.memset(vn[:, :, DH], 1.0)
        nc.gpsimd.memset(vn[:, :, 2 * DH + 1], 1.0)

        # --- projections (emitted in blocks, interleaved into attention) --
        # The V transpose for tile t is deferred (lag-1) so its vt copy has
        # a full block of PE work to land behind; flushed before any q-tile
        # loop that consumes that tile's vn chunks.
        pending_tr = []

        def emit_transposes():
            while pending_tr:
                t, vt = pending_tr.pop(0)
                pt = psum.tile([P, 4, P], BF16, tag="sc", name=f"pt_{t}")
                for s_ in range(4):
                    nc.tensor.transpose(pt[:, s_, :], vt[:, bass.ts(s_, P)],
                                        identity)
                # strided copy drops both heads' dims around the ones cols
                dst = vn[:, bass.ts(t, 4), :].rearrange(
                    "p c (h k) -> p c h k", h=2)[:, :, :, 0:DH]
                src = pt[:].rearrange("p c (h k) -> p c h k", h=2)
                nc.vector.tensor_copy(dst, src)

        def emit_proj(t):
            xt = xpool.tile([P, KD, TT], BF16, name=f"xt_{t}")
            if t == 0:
                # split the very first load so matmuls start sooner
                half = KD // 2
                nc.sync.dma_start(xt[:, 0:half, :],
                                  xT_r[:, 0:half, bass.ts(t, TT)])
                nc.sync.dma_start(xt[:, half:, :],
                                  xT_r[:, half:, bass.ts(t, TT)])
            else:
                nc.sync.dma_start(xt[:], xT_r[:, :, bass.ts(t, TT)])
            uqk = psum.tile([P, 2, TT], F32, tag="sc", name=f"uqk_{t}")
            for k, wsb in ((0, wq_sb), (1, wk_sb)):
                for c in range(KD):
                    nc.tensor.matmul(uqk[:, k, :], wsb[:, c, :], xt[:, c, :],
                                     start=(c == 0), stop=(c == KD - 1))
            nc.vector.tensor_copy(qT[:, bass.ts(t, TT)], uqk[:, 0, :])
            nc.vector.tensor_copy(kT[:, bass.ts(t, TT)], uqk[:, 1, :])
            emit_transposes()
            uv = psum.tile([P, 2, TT], F32, tag="sc", name=f"uv_{t}")
            for c in range(KD):
                nc.tensor.matmul(uv[:, 0, :], wv_sb[:, c, :], xt[:, c, :],
                                 start=(c == 0), stop=(c == KD - 1))
            vt = stage.tile([P, TT], BF16, tag="vt", name=f"vt_{t}")
            nc.scalar.copy(vt[:], uv[:, 0, :])
            pending_tr.append((t, vt))

        emit_proj(0)
        emit_proj(4)

        # --- phase 2: causal attention + interleaved output projection ----
        def pv_step(pvs, jj, ncb, jx, cb, pr):
            b = jx // JB
            c = CB * b + cb
            r = cb - 4 * jj
            lo = P * r if r > 0 else 0
            for h in range(H_LOC):
                nc.tensor.matmul(pvs[(jx, h)][:, lo:],
                                 vn[:, c, bass.ds((DH + 1) * h, DH + 1)],
                                 pr[:, h, lo:],
                                 start=(cb == 0), stop=(cb == ncb - 1))

        def emit_normalize(js, pvs):
            for jx in js:
                jsl = bass.ts(jx, TT)
                for h in range(H_LOC):
                    # custom-DVE recip reads garbage from PSUM; stage to SBUF
                    dn = stage.tile([1, TT], F32, tag="dn",
                                    name=f"dn_{jx}_{h}")
                    nc.vector.tensor_copy(dn[:], pvs[(jx, h)][DH:DH + 1, :])
                    rc = stage.tile([1, TT], F32, tag="rc",
                                    name=f"rc_{jx}_{h}")
                    nc.vector.reciprocal_approx_fast(rc[:], dn[:])
                    rb = stage.tile([DH, TT], F32, tag="rb",
                                    name=f"rb_{jx}_{h}")
                    nc.gpsimd.partition_broadcast(rb[:], rc[:])
                    nc.vector.tensor_mul(
                        attnT[bass.ds(DH * h, DH), jsl],
                        pvs[(jx, h)][0:DH, :], rb[:])

        def emit_wo_unit(jx, fi, ob_eng=None):
            jsl = bass.ts(jx, TT)
            u = psum.tile([P, 2, TT], F32, tag="sc", name=f"wo_{jx}_{fi}")
            for k in range(2):
                nc.tensor.matmul(u[:, k, :], wo_sb[:, bass.ts(2 * fi + k, P)],
                                 attnT[:, jsl], start=True, stop=True)
            ob = obp.tile([P, 2, TT], BF16, tag="ob", name=f"ob_{jx}_{fi}")
            if ob_eng == "scalar":
                nc.scalar.copy(ob[:], u[:])
            else:
                nc.vector.tensor_copy(ob[:], u[:])
            nc.sync.dma_start(outT_r[:, 2 * fi:2 * fi + 2, jsl], ob[:])

        # projection blocks to interleave into each q-tile pair's cb loop:
        # {cb_index: tile}
        proj_sched = {0: {1: 1, 2: 5}, 1: {3: 2, 5: 6}, 2: {4: 3, 8: 7}, 3: {}}

        prev = None  # (js, pvs) of the previous q-tile pair
        for jj in range(JB):
            ncb = 4 * (jj + 1)
            js = (jj, jj + JB)
            emit_transposes()
            wo_work = []
            if prev is not None:
                emit_normalize(*prev)
                wo_work = [(jx, fi) for jx in prev[0] for fi in range(4)]
            pvs = {}
            for jx in js:
                for h in range(H_LOC):
                    pvs[(jx, h)] = psum.tile([DH + 1, TT], F32, tag="a",
                                             bufs=4, name=f"pv_{jx}_{h}")

            pend = {}
            for cb in range(ncb):
                r = cb - 4 * jj
                lo = P * r if r > 0 else 0
                for jx in js:
                    b = jx // JB
                    c = CB * b + cb
                    u = psum.tile([P, 2, TT], F32, tag="sc",
                                  name=f"sc_{jx}_{cb}")
                    for h in range(H_LOC):
                        hp = slice(DH * h, DH * h + DH)
                        nc.tensor.matmul(u[:, h, lo:], kT[hp, bass.ts(c, P)],
                                         qT[hp, bass.ts(jx, TT)][:, lo:],
                                         start=True, stop=True)
                    pr = prp.tile([P, 2, TT], BF16, tag="pr",
                                  name=f"pr_{jx}_{cb}")
                    nc.scalar.activation(pr[:, :, lo:], u[:, :, lo:], EXP,
                                         scale=0.125)
                    if r >= 0:
                        for h in range(H_LOC):
                            nc.gpsimd.affine_select(
                                out=pr[:, h, bass.ts(r, P)],
                                in_=pr[:, h, bass.ts(r, P)],
                                compare_op=mybir.AluOpType.is_ge,
                                fill=0.0,
                                base=0,
                                pattern=[[1, P]],
                                channel_multiplier=-1,
                            )
                    if jx in pend:
                        pv_step(pvs, jj, ncb, jx, *pend[jx])
                    pend[jx] = (cb, pr)
                if cb in proj_sched[jj]:
                    emit_proj(proj_sched[jj][cb])
                if cb >= 1:
                    for _ in range(2):
                        if wo_work:
                            emit_wo_unit(*wo_work.pop(0))
            for jx in js:
                pv_step(pvs, jj, ncb, jx, *pend.pop(jx))
            while wo_work:
                emit_wo_unit(*wo_work.pop(0))
            prev = (js, pvs)

        # --- tail: last q-tile pair, per-jx so PE/DVE/Scalar overlap ------
        for jx in prev[0]:
            emit_normalize((jx,), prev[1])
            for fi in range(4):
                emit_wo_unit(jx, fi, ob_eng=("scalar" if fi % 2 else None))

        if DEBUG_DUMP:
            dq = nc.dram_tensor("dbg_qT", [P, T], BF16,
                                kind="ExternalOutput").ap()
            dk = nc.dram_tensor("dbg_kT", [P, T], BF16,
                                kind="ExternalOutput").ap()
            dv = nc.dram_tensor("dbg_vn", [P, NCH, 2 * DH + 2], BF16,
                                kind="ExternalOutput").ap()
            da = nc.dram_tensor("dbg_attnT", [P, T], BF16,
                                kind="ExternalOutput").ap()
            nc.sync.dma_start(dq, qT[:])
            nc.sync.dma_start(dk, kT[:])
            nc.sync.dma_start(dv, vn[:])
            nc.sync.dma_start(da, attnT[:])


_NC_CACHE = None


def _get_nc():
    global _NC_CACHE
    if _NC_CACHE is None:
        nc = bacc.Bacc("TRN2", target_bir_lowering=False, debug=False,
                       num_devices=N_CORES)
        with tile.TileContext(nc) as tc:
            _body(tc)
        nc.compile()
        _NC_CACHE = nc
    return _NC_CACHE


_BF = ml_dtypes.bfloat16


def _in_maps(x, W_Q, W_K, W_V, W_O):
    xT = np.ascontiguousarray(
        np.asarray(x, dtype=np.float32).reshape(T, D).T).astype(_BF)
    W_Q = np.asarray(W_Q, dtype=np.float32)
    W_K = np.asarray(W_K, dtype=np.float32)
    W_V = np.asarray(W_V, dtype=np.float32)
    W_O = np.asarray(W_O, dtype=np.float32)
    maps = []
    for i in range(N_CORES):
        sl = slice(P * i, P * i + P)
        maps.append({
            "xT": xT,
            "wq": np.ascontiguousarray(W_Q[:, sl]).astype(_BF),
            "wk": np.ascontiguousarray(W_K[:, sl]).astype(_BF),
            "wv": np.ascontiguousarray(W_V[:, sl]).astype(_BF),
            "wo": np.ascontiguousarray(W_O[sl, :]).astype(_BF),
        })
    return maps


def _gather(results):
    acc = np.zeros([D, T], np.float32)
    for r in results:
        acc += r["outT"].astype(np.float32)
    return np.ascontiguousarray(acc.T).reshape(B, S, D)


def kernel(x, W_Q, W_K, W_V, W_O):
    nc = _get_nc()
    res = run_bass_kernel_spmd(nc, _in_maps(x, W_Q, W_K, W_V, W_O),
                               core_ids=list(range(N_CORES)))
    return _gather(res.results)


def kernel_profiled(x, W_Q, W_K, W_V, W_O):
    """Like kernel() but with NTFF tracing.

    Returns (output, exec_time_ns, insts) — insts is the annotated
    gauge instruction list for the traced core (or None).
    """
    nc = _get_nc()
    res = run_bass_kernel_spmd(nc, _in_maps(x, W_Q, W_K, W_V, W_O),
                               core_ids=list(range(N_CORES)), trace=True)
    insts = None
    if res.instructions_and_trace is not None:
        insts = res.instructions_and_trace[0]
    return _gather(res.results), res.exec_time_ns, insts
